# revision 17
# baseline (speedup 1.0000x reference)
"""ChebyKAN linear layer on 8 Trainium2 NeuronCores.

Math: y[b,j] = sum_{i,k} T_k(tanh(x[b,i])) * C[i,j,k],  k = 0..8.

  - Device computes the PRODUCT basis Q = [T1, T1^2, T1*T2, T2^2, T2*T3,
    T3^2, T3*T4, T4^2] as bf16 tiles straight off the engines (no DMA
    casts).  ACT: B1=tanh(x), B2=Sq(B1), B4=Sq(2*B2-1), B8=Sq(2*B4-1),
    B6=Sq(t3); DVE: t2=2*B2-1, u=4*B2-3, B3=B1*t2, t3=B1*u, t4=2*B4-1,
    B5=t2*t3, B7=t3*t4.  B2 doubles as the chain value t2a and B4 as
    t4a, so the whole basis is 12 whole-superchunk [128,2048] ops.
    Since T_2m = 2*Q_2m - 1 and T_2m+1 = 2*Q_2m+1 - T1, affine
    corrections fold into host-side coefficients:
       A_1 = C_1 - C_3 - C_5 - C_7,  A_k = 2*C_k (k>=2),
       bias_j = sum_i (C_0 - C_2 - C_4 - C_6 - C_8)[i,j]
    (bias added during PSUM eviction).  x ships as bf16 (tanh input
    rounding is within budget; rel err ~1.1e-2 vs the 2e-2 gate).
  - The k>=1 contraction is a (2048 x 4096) @ (4096 x 512) bf16 matmul
    per core: 512 matmuls of [128i x 128b] @ [128i x 512j], fp32 PSUM,
    fwl weight loads, steady 216 ns/matmul = the N=512 streaming floor.
    Per-bank psum tiles (bufs=8) keep evictions off the matmul critical
    path; the last superchunk runs bc-outer so stores pipeline into the
    kernel tail.
  - Ramp engineering (the DMA rings deliver their first transfers only
    ~2.5 us after the queue instruction, and the PE HAM clock-gate needs
    a continuous ~2-window busy burst to reach 2.4 GHz):
      * 37 zero-data warmup matmuls bridge queue-open to data-ready,
      * x superchunk-0 arrives in 4 slices split across both HWDGE
        rings, interleaved with the first coefficient chunks,
      * the ACT table load runs behind only the 3 scalar-ring DMA
        enqueues, off the first-tanh critical path,
      * superchunk-0 B2 slices 0/1 are produced on the idle DVE.
  - Consumption order matches production completion order:
    K_ORDER = [1, 2, 4, 3, 8, 5, 6, 7].
  - SBUF note: the coefficient tile is allocated first -- a base offset
    that is not 16B-aligned costs +43 ns on every matmul's moving-
    operand stream (measured).  Scratch allocations are 16B-padded.

Sharding: data-parallel over Bv (16384 -> 8 x 2048), cheby_coeffs
replicated (host-relaid-out, bf16). Host pre-transposes x so the
contraction index i lands on SBUF partitions.
"""

import json as _json

import numpy as np

# ---------------------------------------------------------------------------
# Container workarounds (inlined so kernel.py is self-contained):
#  1. walrus here refuses instructions carrying >1 sem-wait; hoist excess
#     waits onto NoOps inserted before the offender (same engine queue).
#  2. TileContext tail drain accumulates one wait per logical processor;
#     pre-split them the same way.
# ---------------------------------------------------------------------------

import concourse.bass as bass
import concourse.tile as tile
from concourse import mybir
from concourse._compat import with_exitstack
from concourse.bass_utils import run_bass_kernel_spmd
from concourse.vector_clock import ScopedClock, VectorClock

_MAX_WAITS = 1


def _legalize_bir_json(raw: bytes) -> bytes:
    bir = _json.loads(raw)
    changed = False
    for fn in bir.get("functions", []):
        for blk in fn.get("blocks", []):
            out = []
            for inst in blk.get("instructions", []):
                si = inst.get("sync_info")
                waits = (si or {}).get("on_wait") or []
                if len(waits) > _MAX_WAITS:
                    changed = True
                    excess, keep = waits[:-_MAX_WAITS], waits[-_MAX_WAITS:]
                    for j, w in enumerate(excess):
                        out.append(
                            {
                                "debug": inst.get("debug", 0),
                                "engine": inst["engine"],
                                "ins": [],
                                "name": f"{inst['name']}--w{j}",
                                "opcode": "NoOp",
                                "outs": [],
                                "sync_info": {"on_update": [], "on_wait": [w]},
                                "text_hint": "wait_split",
                            }
                        )
                    si["on_wait"] = keep
                out.append(inst)
            blk["instructions"] = out
    return _json.dumps(bir).encode() if changed else raw


def _patched_drain_and_barrier(self, tick_clock, wait_clock):
    gc = tick_clock.global_clock
    n = len(gc)
    for proc in range(n):
        t = gc[proc]
        if t <= 0:
            continue
        vec = [0] * n
        vec[proc] = t
        nop = self.nc.sync.nop(nofuse=True, hint="tail_drain_split")
        wait_clock.add_sem_waits(nop.ins, ScopedClock({None: VectorClock(vec)}))
    self.nc.sync.drain()
    self.nc.all_engine_barrier()
    assert self.sems is not None
    popped = self.nc._tile_sem_poison_stack.pop()
    assert popped is self._sem_poison
    self.nc.clear_and_free_semaphores(list(self.sems.allocated().values()))
    self.nc.all_engine_barrier()


def _apply_patches():
    if getattr(bass.Bass, "_cheby_patched", False):
        return
    orig = bass.Bass.to_json_bytes

    def patched(self, *a, **kw):
        return _legalize_bir_json(orig(self, *a, **kw))

    bass.Bass.to_json_bytes = patched
    tile.TileContext._drain_and_barrier = _patched_drain_and_barrier
    bass.Bass._cheby_patched = True


_apply_patches()

# ---------------------------------------------------------------------------
# Problem constants (hardcoded per the harness contract)
# ---------------------------------------------------------------------------
NCORES = 8
BV, DIM, K = 16384, 512, 9
BC = BV // NCORES          # 2048 rows per core
SC = 512                   # b-superchunk width
NSC = BC // SC             # 4 superchunks per core
NIC = DIM // 128           # 4 i-chunks
NCH = NIC * (K - 1)        # 32 contraction chunks, c -> (K_ORDER[c//4], c%4)
W = NIC * SC               # 2048: whole-superchunk free width
K_ORDER = [1, 2, 4, 3, 8, 5, 6, 7]   # consumption order = production completion order
NWARM = 37                 # HAM warmup matmuls

F32 = mybir.dt.float32
BF16 = mybir.dt.bfloat16
AFT = mybir.ActivationFunctionType
ALU = mybir.AluOpType


def _build_nc():
    nc = bass.Bass()
    xt_d = nc.dram_tensor("xt", (128, NSC, W), BF16, kind="ExternalInput")
    cm_d = nc.dram_tensor("cmat", (128, NCH * DIM), BF16, kind="ExternalInput")
    bi_d = nc.dram_tensor("bias", (128, DIM), F32, kind="ExternalInput")
    y_d = nc.dram_tensor("y", (128, NSC, NSC * DIM), F32, kind="ExternalOutput")

    @with_exitstack
    def kern(ctx, tc):
        nc = tc.nc
        cpool = ctx.enter_context(tc.tile_pool(name="cmat", bufs=1))
        spool = ctx.enter_context(tc.tile_pool(name="scratch", bufs=1))
        xpool = ctx.enter_context(tc.tile_pool(name="x", bufs=2))
        upool = ctx.enter_context(tc.tile_pool(name="u", bufs=1))
        bpool = ctx.enter_context(tc.tile_pool(name="basis", bufs=2))
        ppool = ctx.enter_context(tc.tile_pool(name="ps", bufs=8, space="PSUM"))
        ypool = ctx.enter_context(tc.tile_pool(name="y", bufs=2))

        # --- sync queue: x superchunk-0 slices interleaved with the first
        # coefficient chunks (k-major per K_ORDER), then the bulk groups.
        # cm is allocated FIRST so its SBUF base stays aligned — an
        # unaligned coefficient base measurably slows every matmul's
        # moving-operand stream (+43 ns/MM observed).
        cm = cpool.tile([128, NCH * DIM], BF16, tag="cmat")
        # warmup scratch memsets lead the gpsimd queue so the HAM warmup
        # matmuls can start as soon as the queues open.
        wstat = spool.tile([128, 8], BF16, tag="wstat")
        wmov = spool.tile([128, 128], BF16, tag="wmov")
        nc.gpsimd.memset(wstat[:], 0.0)
        nc.gpsimd.memset(wmov[:], 0.0)
        negone4 = spool.tile([128, 4], F32, tag="negone")  # 16B-padded
        nc.gpsimd.memset(negone4[:], -1.0)
        negone = negone4[:, 0:1]
        x0 = xpool.tile([128, W], BF16, tag="x", name="x_0")

        def xsl(ic):
            return slice(ic * SC, (ic + 1) * SC)

        # scalar ring: x0 even slices + cm chunk 1 (before the first ACT,
        # so both HWDGE rings stream the k=1 working set in parallel);
        # sync ring: the rest, k-major.  DMA transfers lag their queue
        # instruction by ~2.5 us of ring startup, so supply on two rings
        # is what keeps the first k=1/k=2 matmul groups fed.
        nc.scalar.dma_start(x0[:, xsl(1)], xt_d[:, 0, xsl(1)])
        nc.scalar.dma_start(x0[:, xsl(2)], xt_d[:, 0, xsl(2)])
        nc.scalar.dma_start(x0[:, xsl(3)], xt_d[:, 0, xsl(3)])
        nc.sync.dma_start(x0[:, xsl(0)], xt_d[:, 0, xsl(0)])
        nc.sync.dma_start(cm[:, 0:DIM], cm_d[:, 0:DIM])
        nc.sync.dma_start(cm[:, DIM : 2 * DIM], cm_d[:, DIM : 2 * DIM])
        nc.sync.dma_start(cm[:, 2 * DIM : 4 * DIM], cm_d[:, 2 * DIM : 4 * DIM])
        for g in range(1, NCH // 4):
            lo, hi = g * 4 * DIM, (g + 1) * 4 * DIM
            nc.sync.dma_start(cm[:, lo:hi], cm_d[:, lo:hi])
        bi = ypool.tile([128, DIM], F32, tag="bias")
        nc.sync.dma_start(bi[:], bi_d[:])

        def cmt(c):
            return cm[:, c * DIM : (c + 1) * DIM]

        # --- HAM warmup: zero matmuls keep the PE activity monitor busy
        # while the first basis tiles are produced, so the real stream
        # reaches the warm 2.4 GHz issue rate sooner.
        pss = [
            [
                ppool.tile([128, DIM], F32, tag="ps", name=f"ps_{s}_{bc}")
                for bc in range(NSC)
            ]
            for s in range(NSC)
        ]
        for w in range(NWARM):
            nc.tensor.matmul(
                pss[0][0][0:8, 0:128], wstat[:], wmov[:], start=True, stop=True
            )

        for s in range(NSC):
            last = s == NSC - 1
            if s == 0:
                xt = x0
            else:
                xt = xpool.tile([128, W], BF16, tag="x", name=f"x_{s}")
                nc.sync.dma_start(xt[:], xt_d[:, s, :])

            # basis + chain tiles for this superchunk (all bf16; engine
            # math is f32 internally). B2 doubles as t2a, B4 as t4a.
            B = {
                k: bpool.tile([128, W], BF16, tag=f"B{k}", name=f"B{s}_{k}")
                for k in range(1, 9)
            }

            def ctile(tag):
                return upool.tile([128, W], BF16, tag=tag, name=f"{tag}_{s}")

            t2 = ctile("t2")
            u = ctile("u")
            t3 = ctile("t3")
            t4 = ctile("t4")

            # ACT: B1 (gates the PE k=1 chunks), B2, B4, B8 | B6 after t3
            if s == 0:
                for ic in range(NIC):
                    nc.scalar.activation(B[1][:, xsl(ic)], xt[:, xsl(ic)], AFT.Tanh)
                # first two B2 slices on the otherwise-idle DVE so the k=2
                # matmul group starts right behind k=1
                for ic in range(2):
                    nc.vector.tensor_mul(
                        B[2][:, xsl(ic)], B[1][:, xsl(ic)], B[1][:, xsl(ic)]
                    )
                for ic in range(2, NIC):
                    nc.scalar.activation(
                        B[2][:, xsl(ic)], B[1][:, xsl(ic)], AFT.Square
                    )
            else:
                nc.scalar.activation(B[1][:], xt[:], AFT.Tanh)
                nc.scalar.activation(B[2][:], B[1][:], AFT.Square)
            nc.scalar.activation(
                B[4][:], B[2][:], AFT.Square, scale=2.0, bias=negone
            )  # T2^2 = Sq(2*B2 - 1)
            nc.scalar.activation(
                B[8][:], B[4][:], AFT.Square, scale=2.0, bias=negone
            )  # T4^2 = Sq(2*B4 - 1)

            # DVE chain
            nc.vector.tensor_scalar(t2[:], B[2][:], 2.0, 1.0, ALU.mult, ALU.subtract)
            nc.vector.tensor_scalar(u[:], B[2][:], 4.0, 3.0, ALU.mult, ALU.subtract)
            nc.vector.tensor_mul(B[3][:], B[1][:], t2[:])   # T1*T2
            nc.vector.tensor_mul(t3[:], B[1][:], u[:])      # T3 = T1*(4*t2a-3)
            nc.vector.tensor_scalar(t4[:], B[4][:], 2.0, 1.0, ALU.mult, ALU.subtract)
            nc.vector.tensor_mul(B[5][:], t2[:], t3[:])     # T2*T3
            nc.scalar.activation(B[6][:], t3[:], AFT.Square)
            nc.vector.tensor_mul(B[7][:], t3[:], t4[:])     # T3*T4

            # evict previous superchunk (runs while this one's matmuls go)
            if s >= 1:
                yt = ypool.tile([128, NSC * DIM], F32, tag="y", name=f"y_{s - 1}")
                for bc in range(NSC):
                    nc.vector.tensor_add(
                        yt[:, bc * DIM : (bc + 1) * DIM], pss[s - 1][bc][:], bi[:]
                    )
                nc.sync.dma_start(y_d[:, s - 1, :], yt[:])

            if not last:
                # c-outer / bc-inner: psum banks accumulate in lockstep so
                # the PE consumes each basis tile the moment it is produced
                for c in range(NCH):
                    k, ic = K_ORDER[c // NIC], c % NIC
                    for bc in range(NSC):
                        nc.tensor.matmul(
                            pss[s][bc][:],
                            B[k][:, ic * SC + bc * 128 : ic * SC + (bc + 1) * 128],
                            cmt(c),
                            start=(c == 0),
                            stop=(c == NCH - 1),
                        )
            else:
                # last superchunk: bc-outer so evictions + y stores pipeline
                # into the kernel tail
                for bc in range(NSC):
                    for c in range(NCH):
                        k, ic = K_ORDER[c // NIC], c % NIC
                        nc.tensor.matmul(
                            pss[s][bc][:],
                            B[k][:, ic * SC + bc * 128 : ic * SC + (bc + 1) * 128],
                            cmt(c),
                            start=(c == 0),
                            stop=(c == NCH - 1),
                        )
                    yt = ypool.tile([128, DIM], F32, tag="ylast", name=f"yl_{bc}")
                    nc.vector.tensor_add(yt[:], pss[s][bc][:], bi[:])
                    nc.sync.dma_start(
                        y_d[:, s, bc * DIM : (bc + 1) * DIM], yt[:]
                    )

    with tile.TileContext(nc) as tc:
        kern(tc)
    return nc


_NC_CACHE = None


def _get_nc():
    global _NC_CACHE
    if _NC_CACHE is None:
        _NC_CACHE = _build_nc()
    return _NC_CACHE


def _prep_inputs(x, cheby_coeffs):
    import ml_dtypes

    C = np.asarray(cheby_coeffs, dtype=np.float32)
    # product-basis coefficient transform (see module docstring)
    A = np.empty((DIM, DIM, K - 1), np.float32)
    A[:, :, 0] = C[:, :, 1] - C[:, :, 3] - C[:, :, 5] - C[:, :, 7]
    for k in range(2, K):
        A[:, :, k - 1] = 2.0 * C[:, :, k]
    bias_j = (
        (C[:, :, 0] - C[:, :, 2] - C[:, :, 4] - C[:, :, 6] - C[:, :, 8])
        .sum(axis=0, dtype=np.float64)
        .astype(np.float32)
    )
    bias = np.ascontiguousarray(np.broadcast_to(bias_j, (128, DIM)))
    # contraction chunk c = idx*4 + ic holds A[ic*128:(ic+1)*128, :, K_ORDER[idx]];
    # flat layout (128, NCH*DIM), contiguous per partition line
    cmat = np.empty((NCH, 128, DIM), np.float32)
    for idx, k in enumerate(K_ORDER):
        for ic in range(NIC):
            cmat[idx * NIC + ic] = A[ic * 128 : (ic + 1) * 128, :, k - 1]
    cm2 = np.ascontiguousarray(cmat.transpose(1, 0, 2).reshape(128, NCH * DIM)).astype(
        ml_dtypes.bfloat16
    )
    # x layout (128, NSC, NIC*SC): [p, s, ic*SC+b], i = ic*128 + p
    xc = (
        np.asarray(x, dtype=np.float32)
        .astype(ml_dtypes.bfloat16)
        .reshape(NCORES, BC, NIC, 128)
    )
    in_maps = []
    for c in range(NCORES):
        # (BC, NIC, 128) -> [p, s, ic, b]
        xv = xc[c].reshape(NSC, SC, NIC, 128).transpose(3, 0, 2, 1)
        in_maps.append(
            {
                "xt": np.ascontiguousarray(xv).reshape(128, NSC, W),
                "cmat": cm2,
                "bias": bias,
            }
        )
    return in_maps


def kernel(x, cheby_coeffs, _trace=False, _tmpdir=None):
    nc = _get_nc()
    in_maps = _prep_inputs(x, cheby_coeffs)
    res = run_bass_kernel_spmd(
        nc,
        in_maps,
        core_ids=list(range(NCORES)),
        trace=_trace,
        tmpdir=_tmpdir,
    )
    # y layout per core: [p, s, bc*DIM + j] = y[b = s*512 + bc*128 + p, j]
    y = np.concatenate(
        [
            r["y"].reshape(128, NSC, NSC, DIM).transpose(1, 2, 0, 3).reshape(BC, DIM)
            for r in res.results
        ],
        axis=0,
    )
    if _trace:
        kernel.last_result = res
    return y
